# revision 28
# baseline (speedup 1.0000x reference)
"""Trainium2 Bass kernel for nn_Attention_LoRA_FFT.

Sharding: data-parallel over batch B=8 across the 8 NeuronCores; the DCT
LoRA weight reconstruction (Bm.T @ S.T @ Bm) is replicated on each core.

Per-core device program (everything float32r on the PE, fp32 accumulate):
  A) G = Sk.T @ Bm ; WkT = Bm.T @ G          (same for Sv -> WvT)
  B) kT = W_k_qkv @ x.T + WkT-apply          ([feat, tok] layout)
     qT = W_q_qkv @ x.T
     V' = [x @ W_v_qkv.T + x @ Wv.T | 1]     ([tok, feat] layout + ones col)
  C) per head: S.T = kT_h-slices.T @ qT_h    (K=64 matmuls, head pairs share
                                              the PE via partition packing)
     P.T = exp(S.T * hd^-0.5)                (ACT; no max-subtraction: scores
                                              are O(10), exp cannot overflow)
     [O.T ; Z] = V'.T @ P.T                  (ones column gives softmax denom)
     O.T *= broadcast(1/Z)                   (K=1 matmul broadcast + DVE mul)
  D) y.T = W_proj @ O.T + b                  -> DMA out, host transposes

SBUF pools live on two allocation stacks (left/right); alloc/release order
below is chosen so each side pops LIFO and the per-partition peak stays
under the ~208 KiB budget.
"""

import os
import sys

for _p in ("/opt/trn_rl_repo", "/root/.axon_site/_ro/trn_rl_repo"):
    if os.path.isdir(_p) and _p not in sys.path:
        sys.path.insert(0, _p)

import numpy as np

import concourse.bacc as bacc
import concourse.mybir as mybir
from concourse.tile import TileContext
from concourse.bass_utils import run_bass_kernel_spmd

B, N, C = 8, 1024, 1024
H, HD = 16, 64
NCORES = 8
PC = C // 128  # 8 partition chunks per 1024 dim
F32 = mybir.dt.float32
F32R = mybir.dt.float32r
BF16 = mybir.dt.bfloat16
EXP = mybir.ActivationFunctionType.Exp


def _dct_matrix(n: int) -> np.ndarray:
    i = np.arange(n, dtype=np.float32)[:, None]
    j = np.arange(n, dtype=np.float32)[None, :]
    m = np.sqrt(np.float32(2.0 / n)) * np.cos(
        np.float32(np.pi) * i * (2.0 * j + 1.0) / np.float32(2.0 * n)
    )
    m[0, :] = np.sqrt(np.float32(1.0 / n))
    return m.astype(np.float32)


def _build():
    nc = bacc.Bacc("TRN2", target_bir_lowering=False, debug=False, num_devices=NCORES)

    xT_d = nc.dram_tensor("xT", [C, N], F32R, kind="ExternalInput")
    wqkvT_d = nc.dram_tensor("wqkvT", [C, 3 * C], F32R, kind="ExternalInput")
    wprojT_d = nc.dram_tensor("wprojT", [C, C], F32R, kind="ExternalInput")
    bias_d = nc.dram_tensor("bias", [C, 1], F32, kind="ExternalInput")
    bm_d = nc.dram_tensor("bm", [C, C], F32R, kind="ExternalInput")
    sw_d = nc.dram_tensor("sw", [C, C], F32R, kind="ExternalInput")
    bmq_d = nc.dram_tensor("bmq", [C, 256], F32R, kind="ExternalInput")
    yT_d = nc.dram_tensor("yT", [C, N], F32, kind="ExternalOutput")
    cc_in = nc.dram_tensor("cc_in", [C, 256], F32R)
    cc_out = nc.dram_tensor("cc_out", [NCORES * C, 256], F32R, addr_space="Shared")

    def chunked(dram_ap, sb):
        """DMA a [C, F] DRAM AP into a [128, PC, F] SBUF tile, one DMA per
        128-row chunk so downstream consumers can start early."""
        for cc in range(PC):
            nc.sync.dma_start(
                out=sb[:, cc, :], in_=dram_ap[cc * 128 : (cc + 1) * 128, :]
            )

    def col_slab(dram_ap, pool, tag, f0, width):
        """[128, PC, width] slab of columns f0:f0+width of a [C, F] DRAM AP."""
        slab = pool.tile([128, PC, width], F32R, tag=tag, name=tag)
        nc.sync.dma_start(
            out=slab[:],
            in_=dram_ap[:, f0 : f0 + width].rearrange("(cc p) f -> p cc f", p=128),
        )
        return slab

    with TileContext(nc) as tc:
        # ---- left stack: small consts, then wv, wk, x (popped end of B) ----
        small_p = tc.alloc_tile_pool(name="small", bufs=1, side="left")
        ones_sb = small_p.tile([1, 128], F32R, tag="ones")
        nc.vector.memset(ones_sb[:].bitcast(F32), 1.0)
        bias_sb = small_p.tile([128, PC, 1], F32, tag="bias")
        nc.sync.dma_start(
            out=bias_sb[:], in_=bias_d.rearrange("(cc p) o -> p cc o", p=128)
        )

        wv_p = tc.alloc_tile_pool(name="wvp", bufs=1, side="left")
        wk_p = tc.alloc_tile_pool(name="wkp", bufs=1, side="left")
        x_p = tc.alloc_tile_pool(name="xp", bufs=1, side="left")
        wv_sb = wv_p.tile([128, PC, C], F32R, tag="wv")
        wk_sb = wk_p.tile([128, PC, C], F32R, tag="wk")
        x_sb = x_p.tile([128, PC, N], F32R, tag="x")
        chunked(xT_d, x_sb)

        # ================= Phase A: LoRA weight reconstruction ========
        # Sharded across the 8 cores: each core builds a 256-column slice of
        # one weight (cores 0-3: WkT quarters, 4-7: WvT quarters — chosen by
        # per-core input data sw/bmq, the program is identical), then an
        # AllGather distributes the full WkT+WvT to every core.
        slabA_p = tc.alloc_tile_pool(name="slabA", bufs=3, side="right")
        bm_p = tc.alloc_tile_pool(name="bmp", bufs=1, side="right")
        bmq_p = tc.alloc_tile_pool(name="bmqp", bufs=1, side="right")
        g_p = tc.alloc_tile_pool(name="gp", bufs=1, side="right")
        wpart_p = tc.alloc_tile_pool(name="wpartp", bufs=1, side="right")
        psA = tc.alloc_tile_pool(name="psA", bufs=4, space="PSUM")

        bm_sb = bm_p.tile([128, PC, C], F32R, tag="bm")
        chunked(bm_d, bm_sb)
        bmq_sb = bmq_p.tile([128, PC, 256], F32R, tag="bmq")
        nc.sync.dma_start(
            out=bmq_sb[:], in_=bmq_d.rearrange("(cc p) f -> p cc f", p=128)
        )

        _phase_A = nc.named_scope("phaseA"); _phase_A.__enter__()
        g_sb = g_p.tile([128, PC, 256], F32R, tag="g", name="g_sb")
        wpart_sb = wpart_p.tile([128, PC, 256], F32R, tag="wpart", name="wpart_sb")
        # step 1: G[a, fq] = sum_b S[b, a] * Bmq[b, fq]   (256-col slice)
        for at in range(PC):
            slab = col_slab(sw_d, slabA_p, "slabA", at * 128, 128)
            ps = psA.tile([128, 256], F32, tag="psA", name="psA_t")
            for bc in range(PC):
                nc.tensor.matmul(
                    ps[:],
                    slab[:, bc, :],
                    bmq_sb[:, bc, :],
                    start=(bc == 0),
                    stop=(bc == PC - 1),
                )
            nc.scalar.copy(g_sb[:, at, :], ps[:])
        # step 2: WT[c, fq] = sum_a Bm[a, c] * G[a, fq]
        for ct in range(PC):
            ps = psA.tile([128, 256], F32, tag="psA", name="psA_t")
            for ac in range(PC):
                nc.tensor.matmul(
                    ps[:],
                    bm_sb[:, ac, ct * 128 : (ct + 1) * 128],
                    g_sb[:, ac, :],
                    start=(ac == 0),
                    stop=(ac == PC - 1),
                )
            nc.scalar.copy(wpart_sb[:, ct, :], ps[:])
            nc.sync.dma_start(
                out=cc_in[ct * 128 : (ct + 1) * 128, :], in_=wpart_sb[:, ct, :]
            )
        _phase_A.__exit__(None, None, None)
        psA.release()
        wpart_p.release()
        g_p.release()
        bmq_p.release()
        bm_p.release()
        slabA_p.release()

        # ================= Phase B: qkv + lora apply ==================
        kt_p = tc.alloc_tile_pool(name="ktp", bufs=1, side="right")
        qt_p = tc.alloc_tile_pool(name="qtp", bufs=1, side="right")
        vp_p = tc.alloc_tile_pool(name="vpp", bufs=1, side="right")
        slabB_p = tc.alloc_tile_pool(name="slabB", bufs=3, side="right")
        psB = tc.alloc_tile_pool(name="psB", bufs=4, space="PSUM")

        kT_sb = kt_p.tile([128, PC, N], BF16, tag="kT")
        qT_sb = qt_p.tile([128, PC, N], BF16, tag="qT")
        vp_sb = vp_p.tile([128, PC, H, HD + 1], BF16, tag="vp")

        _phase_B = nc.named_scope("phaseB"); _phase_B.__enter__()
        # ---- qT = W_q @ x.T  [feat, tok] (no lora dep: covers the gather) --
        for fc in range(PC):
            slab = col_slab(wqkvT_d, slabB_p, "slabB", fc * 128, 128)
            for th in range(2):
                ps = psB.tile([128, 512], F32, tag="psB", name="psB_t")
                for cc in range(PC):
                    nc.tensor.matmul(
                        ps[:],
                        slab[:, cc, :],
                        x_sb[:, cc, th * 512 : (th + 1) * 512],
                        start=(cc == 0),
                        stop=(cc == PC - 1),
                    )
                nc.scalar.copy(qT_sb[:, fc, th * 512 : (th + 1) * 512], ps[:])

        # AllGather the WkT/WvT quarters while qT runs, then read back.
        nc.gpsimd.collective_compute(
            "AllGather",
            mybir.AluOpType.bypass,
            replica_groups=[list(range(NCORES))],
            ins=[cc_in[:]],
            outs=[cc_out[:]],
        )
        for wi, w_sb in ((0, wk_sb), (1, wv_sb)):
            for fq in range(4):
                base = (wi * 4 + fq) * C
                for cc in range(PC):
                    nc.gpsimd.dma_start(
                        out=w_sb[:, cc, fq * 256 : (fq + 1) * 256],
                        in_=cc_out[base + cc * 128 : base + (cc + 1) * 128, :],
                    )

        # ---- kT = W_k @ x.T + Wk-lora @ x.T  [feat, tok] ----
        for fc in range(PC):
            slab = col_slab(wqkvT_d, slabB_p, "slabB", C + fc * 128, 128)
            for th in range(2):
                ps = psB.tile([128, 512], F32, tag="psB", name="psB_t")
                for cc in range(PC):
                    nc.tensor.matmul(
                        ps[:],
                        slab[:, cc, :],
                        x_sb[:, cc, th * 512 : (th + 1) * 512],
                        start=(cc == 0),
                        stop=False,
                    )
                for cc in range(PC):
                    nc.tensor.matmul(
                        ps[:],
                        wk_sb[:, cc, fc * 128 : (fc + 1) * 128],
                        x_sb[:, cc, th * 512 : (th + 1) * 512],
                        start=False,
                        stop=(cc == PC - 1),
                    )
                nc.scalar.copy(kT_sb[:, fc, th * 512 : (th + 1) * 512], ps[:])
        slabB_p.release()

        # ---- V' = x @ W_v.T + x @ Wv-lora.T  [tok, feat | 1] ----
        vslab_p = tc.alloc_tile_pool(name="vslab", bufs=2, side="right")
        for fh in range(2):
            vslab = col_slab(wqkvT_d, vslab_p, "vslab", 2 * C + fh * 512, 512)
            for tc_i in range(PC):
                ps = psB.tile([128, 512], F32, tag="psB", name="psB_t")
                for cc in range(PC):
                    nc.tensor.matmul(
                        ps[:],
                        x_sb[:, cc, tc_i * 128 : (tc_i + 1) * 128],
                        vslab[:, cc, :],
                        start=(cc == 0),
                        stop=False,
                    )
                for cc in range(PC):
                    nc.tensor.matmul(
                        ps[:],
                        x_sb[:, cc, tc_i * 128 : (tc_i + 1) * 128],
                        wv_sb[:, cc, fh * 512 : (fh + 1) * 512],
                        start=False,
                        stop=(cc == PC - 1),
                    )
                nc.scalar.copy(
                    vp_sb[:, tc_i, fh * 8 : (fh + 1) * 8, 0:HD],
                    ps[:].rearrange("p (h d) -> p h d", d=HD),
                )
        for tc_i in range(PC):
            nc.vector.memset(vp_sb[:, tc_i, :, HD : HD + 1], 1.0)

        _phase_B.__exit__(None, None, None)
        vslab_p.release()
        psB.release()
        x_p.release()
        wk_p.release()
        wv_p.release()

        # ================= Phase C: attention =========================
        ot_p = tc.alloc_tile_pool(name="otp", bufs=1, side="left")
        pt_p = tc.alloc_tile_pool(name="ptp", bufs=2, side="right")
        rz_p = tc.alloc_tile_pool(name="rzp", bufs=2, side="right")
        zb_p = tc.alloc_tile_pool(name="zbp", bufs=2, side="right")
        psS = tc.alloc_tile_pool(name="psS", bufs=1, space="PSUM")
        psO = tc.alloc_tile_pool(name="psO", bufs=1, space="PSUM")

        oT_sb = ot_p.tile([128, PC, N], F32R, tag="oT")
        scale = float(HD) ** -0.5
        _phase_C = nc.named_scope("phaseC"); _phase_C.__enter__()

        # Software pipeline over units (ih, hp): stage 1 (S matmuls + exp on
        # ACT) for unit u runs concurrently with stage 2 (O accumulation +
        # normalization) for unit u-1, so ACT and PE overlap instead of
        # ping-ponging through the in-order PE queue. All attention matmuls
        # are bf16: the two heads of a pair run row-packed (S) in disjoint
        # PE row groups, which also keeps the HAM clock-gate warm.
        units = [(ih, hp) for ih in range(2) for hp in range(H // 2)]
        staged = {}
        ps_big = psS.tile([128, 6, 512], F32, tag="sbig", name="ps_big")
        slot_ctr = [0]

        def stage1(ih, hp):
            i0 = ih * 512
            pts = [
                pt_p.tile([128, PC, 512], BF16, tag=f"pt{sub}", name="pt_t")
                for sub in range(2)
            ]
            for j0 in range(0, PC, 2):  # two j-tiles share one exp
                # manual 3-deep rotation inside one 6-bank psum tile; subtile
                # dep tracking keeps WAR ordering without pool-slot stalls
                slots = []
                for sub in range(2):
                    s = slot_ctr[0] % 3
                    slot_ctr[0] += 1
                    slots.append(ps_big[:, 2 * s : 2 * s + 2, :])
                for dj in range(2):
                    for sub in range(2):  # adjacent row-group pair: concurrent
                        p0 = sub * 64
                        nc.tensor.matmul(
                            slots[sub][:, dj, :],
                            kT_sb[
                                p0 : p0 + 64,
                                hp,
                                (j0 + dj) * 128 : (j0 + dj + 1) * 128,
                            ],
                            qT_sb[p0 : p0 + 64, hp, i0 : i0 + 512],
                        )
                for sub in range(2):
                    nc.scalar.activation(
                        pts[sub][:, j0 : j0 + 2, :].rearrange("p j i -> p (j i)"),
                        slots[sub].rearrange("p j i -> p (j i)"),
                        EXP,
                        scale=scale,
                    )
            staged[(ih, hp)] = pts

        def stage2(ih, hp):
            i0 = ih * 512
            pts = staged.pop((ih, hp))
            for sub in range(2):
                h = 2 * hp + sub
                p0 = sub * 64
                pt = pts[sub]
                ps_o = psO.tile([HD + 1, 512], F32, tag=f"o{sub}", name="psO_t")
                for j in range(PC):
                    nc.tensor.matmul(
                        ps_o[:],
                        vp_sb[:, j, h, :],
                        pt[:, j, :],
                        start=(j == 0),
                        stop=(j == PC - 1),
                    )
                zraw = rz_p.tile([1, 512], F32, tag="rz", name="rz_t")
                nc.vector.tensor_copy(zraw[:], ps_o[HD : HD + 1, :])
                zbc = zb_p.tile([HD, 512], F32, tag="zbc", name="zbc_t")
                nc.gpsimd.partition_broadcast(zbc[:], zraw[:], channels=HD)
                zb = zb_p.tile([HD, 512], F32, tag="zb", name="zb_t")
                nc.vector.reciprocal_approx_fast(zb[:], zbc[:])
                nc.vector.tensor_mul(
                    oT_sb[p0 : p0 + 64, hp, i0 : i0 + 512],
                    ps_o[0:HD, :],
                    zb[:],
                )

        for u in range(len(units) + 1):
            if u < len(units):
                stage1(*units[u])
            if u > 0:
                stage2(*units[u - 1])

        _phase_C.__exit__(None, None, None)
        psO.release()
        psS.release()
        zb_p.release()
        rz_p.release()
        pt_p.release()
        vp_p.release()
        qt_p.release()
        kt_p.release()

        # ================= Phase D: output projection =================
        wp_p = tc.alloc_tile_pool(name="wpp", bufs=1, side="right")
        y_p = tc.alloc_tile_pool(name="yp", bufs=4, side="right")
        psD = tc.alloc_tile_pool(name="psD", bufs=4, space="PSUM")
        _phase_D = nc.named_scope("phaseD"); _phase_D.__enter__()
        wp_sb = wp_p.tile([128, PC, C], F32R, tag="wp")
        chunked(wprojT_d, wp_sb)
        for fo in range(PC):
            for th in range(2):
                ps = psD.tile([128, 512], F32, tag="psD", name="psD_t")
                for cc in range(PC):
                    nc.tensor.matmul(
                        ps[:],
                        wp_sb[:, cc, fo * 128 : (fo + 1) * 128],
                        oT_sb[:, cc, th * 512 : (th + 1) * 512],
                        start=(cc == 0),
                        stop=(cc == PC - 1),
                    )
                y_sb = y_p.tile([128, 512], F32, tag="y", name="y_t")
                nc.vector.tensor_scalar_add(y_sb[:], ps[:], bias_sb[:, fo, :])
                nc.sync.dma_start(
                    out=yT_d[fo * 128 : (fo + 1) * 128, th * 512 : (th + 1) * 512],
                    in_=y_sb[:],
                )
        _phase_D.__exit__(None, None, None)
        psD.release()
        y_p.release()
        wp_p.release()
        ot_p.release()
        small_p.release()

    nc.compile()
    return nc


_CACHE = {}


def _get_nc():
    if "nc" not in _CACHE:
        _CACHE["nc"] = _build()
    return _CACHE["nc"]


def _host_prep(x, W_qkv, W_proj, b_proj, coef_k, coef_v, indices, task):
    x = np.asarray(x, dtype=np.float32)
    W_qkv = np.asarray(W_qkv, dtype=np.float32)
    W_proj = np.asarray(W_proj, dtype=np.float32)
    b_proj = np.asarray(b_proj, dtype=np.float32)
    coef_k = np.asarray(coef_k, dtype=np.float32)
    coef_v = np.asarray(coef_v, dtype=np.float32)
    indices = np.asarray(indices)
    t = int(np.asarray(task).reshape(())) + 1

    assert x.shape == (B, N, C), x.shape

    # Host-side input marshaling: scatter the per-task frequency coefficients
    # into dense C x C planes (the sum across tasks commutes with the linear
    # inverse DCT), exactly as the reference does before its matmuls.
    def scatter(coef, idx):
        s = np.zeros(C * C, dtype=np.float32)
        np.add.at(s, idx.reshape(-1).astype(np.int64), coef.reshape(-1))
        return s.reshape(C, C)

    bm = _dct_matrix(C)
    sk = scatter(coef_k[:t], indices[:t])
    sv = scatter(coef_v[:t], indices[:t])
    shared = {
        "wqkvT": np.ascontiguousarray(W_qkv.T),
        "wprojT": np.ascontiguousarray(W_proj.T),
        "bias": np.ascontiguousarray(b_proj.reshape(C, 1)),
        "bm": bm,
    }
    maps = []
    for b in range(NCORES):
        fq = b % 4
        maps.append(
            {
                "xT": np.ascontiguousarray(x[b].T),
                "sw": sk if b < 4 else sv,
                "bmq": np.ascontiguousarray(bm[:, fq * 256 : (fq + 1) * 256]),
                **shared,
            }
        )
    return maps


def kernel(x, W_qkv, W_proj, b_proj, coef_k, coef_v, indices, task):
    in_maps = _host_prep(x, W_qkv, W_proj, b_proj, coef_k, coef_v, indices, task)
    nc = _get_nc()
    res = run_bass_kernel_spmd(nc, in_maps, list(range(NCORES)))

    out = np.empty((B, N, C), dtype=np.float32)
    for b in range(NCORES):
        out[b] = res.results[b]["yT"].T
    return out
